# revision 12
# baseline (speedup 1.0000x reference)
"""DeTPP assignment loss on Trainium2, data-parallel over batch across 8 NeuronCores.

Pipeline per core (B_shard = 8 batch columns, N_s = 512*8 = 4096 windows):
  host   : pure-index gathers (rolling windows, per-batch row selection,
           true-class logit pick), shard + pack fp16 partition-major layouts
  device : sum(exp) over C=128 (the memory-bound bulk: 4.2 MiB of gathered
           fp16 logits per core), L1/CE cost assembly, exact 24-permutation
           assignment min via pair-sum decomposition, softplus leftover,
           mask-weighted f32 reduction to 128 partial sums
  host   : sum partials across cores / V

Key algebra: cost[k,t] = base[k,t] + (lse_k - ps_k) with
base = |ot-tt| + |oa-at| - logit[true class]; the (lse_k - ps_k) part is
independent of the assignment, so the 24-perm min runs on `base` alone and
sum_k lse_k + sum_k softplus(ps_k) = ln(prod_k se_k * prod_k (1+e^{ps_k}))
needs a single Ln per window.

Engine layout (from trace analysis): ACT does only exp (+1 ln); the
broadcast-heavy cost/pair-sum chain runs on otherwise-idle GpSimd; DVE does
the sumexp reductions as fp16 halving-tree adds (packed 2-byte tensor_tensor
runs 2x on DVE; tensor_reduce has no fast mode) plus the small tail math.
Chunks are large in the middle, tiny at both ends (fast ramp, short drain).
"""
import numpy as np

L, B, K, C = 2048, 64, 4, 128
I = 512
NCORES = 8
BS = B // NCORES          # batch columns per core
NS = I * BS               # windows per core
P = 128                   # partitions
NT = NS // P              # 32 row-tiles per core
KC = K * C                # 512

# tiles per logits DMA chunk: big middle, tiny ends
CHUNKS = [2, 4, 8, 8, 4, 4, 1, 1]
assert sum(CHUNKS) == NT

# small-tensor column offsets within the packed (P, SMW) fp16 tensor
OFF_OLT, OFF_OT, OFF_TT, OFF_OA, OFF_AT, OFF_PS, OFF_M, SMW = \
    0, 512, 640, 768, 896, 1024, 1152, 1184

# ordered-pair column indices (t0*4+t1) for the 6 split assignments:
# (pair handled by k0,k1; complementary pair handled by k2,k3)
SPLITS = [(1, 11), (11, 1), (2, 7), (7, 2), (3, 6), (6, 3)]

_PROGRAM = None


def _prep(in_time, in_amount, in_mcc, out_time, out_amount, out_logits,
          presence, lengths, indices, subset_lengths):
    """Host-side pure-index gather, mirroring reference _windows/_select."""
    f = np.float32
    idx = np.clip(np.asarray(indices), 0, L - 1)            # (I, B)
    br = np.arange(B)[None, :]
    win = (idx[:, :, None] + np.arange(K + 1)[None, None, :]) % L
    bw = br[:, :, None]
    tw = np.asarray(in_time)[win, bw].astype(f)             # (I,B,K+1)
    aw = np.asarray(in_amount)[win, bw].astype(f)
    cw = np.clip(np.asarray(in_mcc)[win, bw], 0, C - 1)     # (I,B,K+1)
    t_true = tw[..., 1:] - tw[..., :1]                      # (I,B,K)
    a_true = aw[..., 1:]
    true_c = cw[..., 1:]
    lg = np.asarray(out_logits)[idx, br].astype(f)          # (I,B,K,C)
    ol_true = np.take_along_axis(lg, true_c[:, :, None, :], axis=3)  # (I,B,K,T)
    ot = np.asarray(out_time)[idx, br].astype(f)            # (I,B,K)
    oa = np.asarray(out_amount)[idx, br].astype(f)
    ps = np.asarray(presence)[idx, br].astype(f)
    m = (np.arange(I)[:, None] < np.asarray(subset_lengths)[None, :]).astype(f)
    return dict(lg=lg, ol_true=ol_true, ot=ot, t_true=t_true, oa=oa,
                a_true=a_true, ps=ps, m=m)


def _pack_core(g, d):
    """Shard batch columns [d*BS, (d+1)*BS) and pack partition-major fp16:
    row n = i*BS + b_local lives at (tile j = n//P, partition p = n%P);
    DRAM layout (P, NT*w) so every DMA is contiguous per partition."""
    sl = slice(d * BS, (d + 1) * BS)

    def pk(a):
        w = int(np.prod(a.shape[2:], dtype=np.int64)) if a.ndim > 2 else 1
        return a[:, sl].reshape(NT, P, w).transpose(1, 0, 2).reshape(P, NT * w)

    small = np.concatenate(
        [pk(g["ol_true"]), pk(g["ot"]), pk(g["t_true"]), pk(g["oa"]),
         pk(g["a_true"]), pk(g["ps"]), pk(g["m"])], axis=1).astype(np.float16)
    assert small.shape == (P, SMW)
    logits = np.ascontiguousarray(pk(g["lg"]).astype(np.float16))
    return {"logits": logits, "small": small}


def _build_program(debug=False):
    import concourse.bacc as bacc
    import concourse.tile as tile
    import concourse.mybir as mybir

    f32 = mybir.dt.float32
    f16 = mybir.dt.float16
    AF = mybir.ActivationFunctionType
    ALU = mybir.AluOpType
    AX = mybir.AxisListType.X

    nc = bacc.Bacc("TRN2", target_bir_lowering=False, debug=debug)
    lg_d = nc.dram_tensor("logits", [P, NT * KC], f16, kind="ExternalInput")
    sm_d = nc.dram_tensor("small", [P, SMW], f16, kind="ExternalInput")
    out_d = nc.dram_tensor("partial", [P, 1], f32, kind="ExternalOutput")

    TS = (P, NT, K, K)

    with tile.TileContext(nc) as tc:
        with tc.tile_pool(name="big", bufs=1) as big, \
             tc.tile_pool(name="res", bufs=1) as res:

            def rtile(tag, shape, dt=f16):
                return res.tile(list(shape), dt, tag=tag, name=tag)

            # warm the DVE micro-op programs used later, in the otherwise
            # idle window before the first chunk lands (first use of an op
            # type costs ~1us extra)
            wrm = rtile("wrm", (P, 4))
            nc.vector.memset(wrm[:], 1.0)
            wr2 = rtile("wr2", (P, 4))
            wr3 = rtile("wr3", (P, 2), f32)
            nc.vector.tensor_add(wr2[:], wrm[:], wrm[:])
            nc.vector.tensor_max(wr2[:], wrm[:], wrm[:])
            nc.vector.tensor_mul(wr2[:], wrm[:], wrm[:])
            nc.vector.tensor_reduce(out=wr3[:, 0:1], in_=wrm[:], axis=AX, op=ALU.mult)
            nc.vector.tensor_reduce(out=wr3[:, 1:2], in_=wrm[:], axis=AX, op=ALU.min)

            # one contiguous DMA for all small per-window inputs, issued on
            # the ACT HWDGE ring so it runs in parallel with chunk 0 on sync
            sm = rtile("sm", (P, SMW))
            nc.scalar.dma_start(out=sm[:], in_=sm_d.ap())
            olt = sm[:, OFF_OLT:OFF_OT].rearrange("p (j a b) -> p j a b", a=K, b=K)
            ot4 = sm[:, OFF_OT:OFF_TT].rearrange("p (j a) -> p j a", a=K)
            tt4 = sm[:, OFF_TT:OFF_OA].rearrange("p (j a) -> p j a", a=K)
            oa4 = sm[:, OFF_OA:OFF_AT].rearrange("p (j a) -> p j a", a=K)
            at4 = sm[:, OFF_AT:OFF_PS].rearrange("p (j a) -> p j a", a=K)
            ps4 = sm[:, OFF_PS:OFF_M].rearrange("p (j a) -> p j a", a=K)
            m1 = sm[:, OFF_M:SMW]

            # --- base[n,k,t] = |ot-tt| + |oa-at| - olt on GpSimd (idle
            # engine; ACT stays exp-only, DVE keeps the reduction stream).
            # abs as max(-x, x) via DVE scalar_tensor_tensor (not lowerable
            # on Pool, and abs_max crashes walrus codegen on fp16). ---
            d_t = rtile("d_t", TS)
            nc.gpsimd.tensor_sub(d_t[:], ot4.unsqueeze(3).broadcast_to(TS),
                                 tt4.unsqueeze(2).broadcast_to(TS))
            d_a = rtile("d_a", TS)
            nc.gpsimd.tensor_sub(d_a[:], oa4.unsqueeze(3).broadcast_to(TS),
                                 at4.unsqueeze(2).broadcast_to(TS))
            # abs = max(x, -x): negate on GpSimd, max on DVE (Pool rejects
            # TensorTensor-max and TensorScalarPtr; DVE stt is slow)
            d_tn = rtile("d_tn", TS)
            nc.gpsimd.tensor_scalar_mul(d_tn[:], d_t[:], -1.0)
            nc.vector.tensor_max(d_t[:], d_t[:], d_tn[:])
            d_an = rtile("d_an", TS)
            nc.gpsimd.tensor_scalar_mul(d_an[:], d_a[:], -1.0)
            nc.vector.tensor_max(d_a[:], d_a[:], d_an[:])
            base = rtile("base", TS)
            nc.gpsimd.tensor_add(base[:], d_t[:], d_a[:])
            nc.gpsimd.tensor_sub(base[:], base[:], olt)

            # pair sums A[t0,t1] = base[k0,t0]+base[k1,t1] (B for k2,k3)
            A = rtile("A", TS)
            nc.gpsimd.tensor_add(A[:], base[:, :, 0, :].unsqueeze(3).broadcast_to(TS),
                                 base[:, :, 1, :].unsqueeze(2).broadcast_to(TS))
            Bp = rtile("Bp", TS)
            nc.gpsimd.tensor_add(Bp[:], base[:, :, 2, :].unsqueeze(3).broadcast_to(TS),
                                 base[:, :, 3, :].unsqueeze(2).broadcast_to(TS))
            mA = rtile("mA", TS)
            nc.vector.tensor_tensor(out=mA[:], in0=A[:],
                                    in1=A[:].transpose([0, 1, 3, 2]), op=ALU.min)
            mB = rtile("mB", TS)
            nc.vector.tensor_tensor(out=mB[:], in0=Bp[:],
                                    in1=Bp[:].transpose([0, 1, 3, 2]), op=ALU.min)
            V6 = rtile("V6", (P, NT, 6))
            for q, (ja, jb) in enumerate(SPLITS):
                a0, a1 = divmod(ja, 4)
                b0, b1 = divmod(jb, 4)
                nc.vector.tensor_add(V6[:, :, q], mA[:, :, a0, a1],
                                     mB[:, :, b0, b1])
            pmin = rtile("pmin", (P, NT), f32)
            nc.vector.tensor_reduce(out=pmin[:], in_=V6[:], axis=AX, op=ALU.min)

            # leftover pieces (tiny): e4 = exp(ps)+1, qe = prod_k e4,
            # pss = sum_k ps
            e4 = rtile("e4", (P, NT, K))
            nc.scalar.activation(out=e4[:], in_=ps4, func=AF.Exp)
            nc.gpsimd.tensor_scalar_add(e4[:], e4[:], 1.0)
            q1 = rtile("q1", (P, NT, 2))
            nc.vector.tensor_mul(q1[:], e4[:, :, 0:2], e4[:, :, 2:4])
            qe = rtile("qe", (P, NT), f32)
            nc.vector.tensor_mul(qe[:], q1[:, :, 0], q1[:, :, 1])
            pss = rtile("pss", (P, NT), f32)
            nc.vector.tensor_reduce(out=pss[:], in_=ps4, axis=AX, op=ALU.add)

            # --- sum(exp) over C per (window, k): fp16 halving-tree adds
            # (packed tensor_tensor = 2x on DVE) + short reduce; graduated
            # chunks; per-chunk product over k folded into the stream ---
            se_all = rtile("se_all", (P, NT, K))
            qs = rtile("qs", (P, NT), f32)
            off = 0
            for ci, t in enumerate(CHUNKS):
                cw = t * KC
                nb = sum(1 for x in CHUNKS if x == t)
                lg = big.tile([P, cw], f16, tag=f"lg{t}", name=f"lg{ci}", bufs=nb)
                nc.sync.dma_start(out=lg[:], in_=lg_d.ap()[:, off * KC:off * KC + cw])
                nc.scalar.activation(out=lg[:], in_=lg[:], func=AF.Exp)
                g = t * K
                v = lg[:].rearrange("p (g c) -> p g c", c=C)
                h1 = big.tile([P, g, 64], f16, tag=f"h1{t}", name=f"h1_{ci}", bufs=nb)
                nc.vector.tensor_add(h1[:], v[:, :, 0:64], v[:, :, 64:128])
                h2 = big.tile([P, g, 32], f16, tag=f"h2{t}", name=f"h2_{ci}", bufs=nb)
                nc.vector.tensor_add(h2[:], h1[:, :, 0:32], h1[:, :, 32:64])
                h3 = big.tile([P, g, 16], f16, tag=f"h3{t}", name=f"h3_{ci}", bufs=nb)
                nc.vector.tensor_add(h3[:], h2[:, :, 0:16], h2[:, :, 16:32])
                with nc.allow_low_precision(reason="sumexp fits fp16"):
                    nc.vector.tensor_reduce(
                        out=se_all[:, off:off + t, :], in_=h3[:],
                        axis=AX, op=ALU.add)
                # qs slice: prod_k sumexp for this chunk's windows
                nc.vector.tensor_reduce(
                    out=qs[:, off:off + t],
                    in_=se_all[:, off:off + t, :], axis=AX, op=ALU.mult)
                off += t

            # ln(prod_k se_k * prod_k (1+e^ps_k)) = sum_k lse_k + softplus sum
            qq = rtile("qq", (P, NT), f32)
            nc.vector.tensor_mul(qq[:], qs[:], qe[:])
            lnq = rtile("lnq", (P, NT), f32)
            nc.scalar.activation(out=lnq[:], in_=qq[:], func=AF.Ln)

            # --- total = (pmin + lnq - pss) * m; partial sums per partition ---
            tot = rtile("tot", (P, NT), f32)
            nc.vector.tensor_add(tot[:], pmin[:], lnq[:])
            nc.vector.tensor_sub(tot[:], tot[:], pss[:])
            totm = rtile("totm", (P, NT), f32)
            nc.vector.tensor_mul(totm[:], tot[:], m1)
            rowsum = rtile("rowsum", (P, 1), f32)
            nc.vector.tensor_reduce(out=rowsum[:], in_=totm[:], axis=AX, op=ALU.add)
            nc.sync.dma_start(out=out_d.ap(), in_=rowsum[:])

    nc.compile()
    return nc


def _get_program():
    global _PROGRAM
    if _PROGRAM is None:
        _PROGRAM = _build_program()
    return _PROGRAM


def kernel(**inputs):
    g = _prep(**inputs)
    in_maps = [_pack_core(g, d) for d in range(NCORES)]
    nc = _get_program()
    from concourse.bass_utils import run_bass_kernel_spmd
    res = run_bass_kernel_spmd(nc, in_maps, list(range(NCORES)))
    total = sum(r["partial"].sum(dtype=np.float64) for r in res.results)
    V = g["m"].sum(dtype=np.float64)
    return np.asarray(np.float32(total) / np.float32(V))


# revision 14
# speedup vs baseline: 1.3415x; 1.3415x over previous
"""DeTPP assignment loss on Trainium2, data-parallel over batch across 8 NeuronCores.

Pipeline per core (B_shard = 8 batch columns, N_s = 512*8 = 4096 windows):
  host   : pure-index gathers (rolling windows, per-batch row selection,
           true-class logit pick), shard + pack fp16 partition-major layouts
  device : sum(exp) over C=128 (the memory-bound bulk: 4.2 MiB of gathered
           fp16 logits per core), L1/CE cost assembly, exact 24-permutation
           assignment min via pair-sum decomposition, softplus leftover,
           mask-weighted f32 reduction to 128 partial sums
  host   : sum partials across cores / V

Key algebra: cost[k,t] = base[k,t] + (lse_k - ps_k) with
base = |ot-tt| + |oa-at| - logit[true class]; the (lse_k - ps_k) part is
independent of the assignment, so the 24-perm min runs on `base` alone and
sum_k lse_k + sum_k softplus(ps_k) = ln(prod_k se_k * prod_k (1+e^{ps_k}))
needs a single Ln per window.

Engine layout (from trace analysis): ACT does only exp (+1 ln); the
broadcast-heavy cost/pair-sum chain runs on otherwise-idle GpSimd; DVE does
the sumexp reductions as fp16 halving-tree adds (packed 2-byte tensor_tensor
runs 2x on DVE; tensor_reduce has no fast mode) plus the small tail math.
Chunks are large in the middle, tiny at both ends (fast ramp, short drain).
"""
import numpy as np

L, B, K, C = 2048, 64, 4, 128
I = 512
NCORES = 8
BS = B // NCORES          # batch columns per core
NS = I * BS               # windows per core
P = 128                   # partitions
NT = NS // P              # 32 row-tiles per core
KC = K * C                # 512

# tiles per logits DMA chunk: big middle, tiny ends
CHUNKS = [2, 4, 8, 8, 4, 4, 1, 1]
assert sum(CHUNKS) == NT

# small-tensor column offsets within the packed (P, SMW) fp16 tensor
OFF_OLT, OFF_OT, OFF_TT, OFF_OA, OFF_AT, OFF_PS, OFF_M, SMW = \
    0, 512, 640, 768, 896, 1024, 1152, 1184

# ordered-pair column indices (t0*4+t1) for the 6 split assignments:
# (pair handled by k0,k1; complementary pair handled by k2,k3)
SPLITS = [(1, 11), (11, 1), (2, 7), (7, 2), (3, 6), (6, 3)]

_PROGRAM = None


def _prep(in_time, in_amount, in_mcc, out_time, out_amount, out_logits,
          presence, lengths, indices, subset_lengths):
    """Host-side pure-index gather, mirroring reference _windows/_select."""
    f = np.float32
    idx = np.clip(np.asarray(indices), 0, L - 1)            # (I, B)
    br = np.arange(B)[None, :]
    win = (idx[:, :, None] + np.arange(K + 1)[None, None, :]) % L
    bw = br[:, :, None]
    tw = np.asarray(in_time)[win, bw].astype(f)             # (I,B,K+1)
    aw = np.asarray(in_amount)[win, bw].astype(f)
    cw = np.clip(np.asarray(in_mcc)[win, bw], 0, C - 1)     # (I,B,K+1)
    t_true = tw[..., 1:] - tw[..., :1]                      # (I,B,K)
    a_true = aw[..., 1:]
    true_c = cw[..., 1:]
    lg = np.asarray(out_logits)[idx, br].astype(f)          # (I,B,K,C)
    ol_true = np.take_along_axis(lg, true_c[:, :, None, :], axis=3)  # (I,B,K,T)
    ot = np.asarray(out_time)[idx, br].astype(f)            # (I,B,K)
    oa = np.asarray(out_amount)[idx, br].astype(f)
    ps = np.asarray(presence)[idx, br].astype(f)
    m = (np.arange(I)[:, None] < np.asarray(subset_lengths)[None, :]).astype(f)
    return dict(lg=lg, ol_true=ol_true, ot=ot, t_true=t_true, oa=oa,
                a_true=a_true, ps=ps, m=m)


def _pack_core(g, d):
    """Shard batch columns [d*BS, (d+1)*BS) and pack partition-major fp16:
    row n = i*BS + b_local lives at (tile j = n//P, partition p = n%P);
    DRAM layout (P, NT*w) so every DMA is contiguous per partition."""
    sl = slice(d * BS, (d + 1) * BS)

    def pk(a):
        w = int(np.prod(a.shape[2:], dtype=np.int64)) if a.ndim > 2 else 1
        return a[:, sl].reshape(NT, P, w).transpose(1, 0, 2).reshape(P, NT * w)

    small = np.concatenate(
        [pk(g["ol_true"]), pk(g["ot"]), pk(g["t_true"]), pk(g["oa"]),
         pk(g["a_true"]), pk(g["ps"]), pk(g["m"])], axis=1).astype(np.float16)
    assert small.shape == (P, SMW)
    logits = np.ascontiguousarray(pk(g["lg"]).astype(np.float16))
    return {"logits": logits, "small": small}


def _build_program(debug=False):
    import concourse.bacc as bacc
    import concourse.tile as tile
    import concourse.mybir as mybir

    f32 = mybir.dt.float32
    f16 = mybir.dt.float16
    AF = mybir.ActivationFunctionType
    ALU = mybir.AluOpType
    AX = mybir.AxisListType.X

    nc = bacc.Bacc("TRN2", target_bir_lowering=False, debug=debug)
    lg_d = nc.dram_tensor("logits", [P, NT * KC], f16, kind="ExternalInput")
    sm_d = nc.dram_tensor("small", [P, SMW], f16, kind="ExternalInput")
    out_d = nc.dram_tensor("partial", [P, 1], f32, kind="ExternalOutput")

    TS = (P, NT, K, K)

    with tile.TileContext(nc) as tc:
        with tc.tile_pool(name="big", bufs=1) as big, \
             tc.tile_pool(name="res", bufs=1) as res:

            def rtile(tag, shape, dt=f16):
                return res.tile(list(shape), dt, tag=tag, name=tag)

            # one contiguous DMA for all small per-window inputs, first on
            # the sync HWDGE so the GpSimd cost chain starts early
            sm = rtile("sm", (P, SMW))
            nc.sync.dma_start(out=sm[:], in_=sm_d.ap())
            ones = rtile("ones", (P, 1))
            nc.vector.memset(ones[:], 1.0)
            olt = sm[:, OFF_OLT:OFF_OT].rearrange("p (j a b) -> p j a b", a=K, b=K)
            ot4 = sm[:, OFF_OT:OFF_TT].rearrange("p (j a) -> p j a", a=K)
            tt4 = sm[:, OFF_TT:OFF_OA].rearrange("p (j a) -> p j a", a=K)
            oa4 = sm[:, OFF_OA:OFF_AT].rearrange("p (j a) -> p j a", a=K)
            at4 = sm[:, OFF_AT:OFF_PS].rearrange("p (j a) -> p j a", a=K)
            ps4 = sm[:, OFF_PS:OFF_M].rearrange("p (j a) -> p j a", a=K)
            m1 = sm[:, OFF_M:SMW]

            # --- base[n,k,t] = |ot-tt| + |oa-at| - olt on GpSimd (idle
            # engine; ACT stays exp-only, DVE keeps the reduction stream).
            # abs as max(-x, x) via DVE scalar_tensor_tensor (not lowerable
            # on Pool, and abs_max crashes walrus codegen on fp16). ---
            # |x| = max(x-y, y-x): both orientations as cheap GpSimd
            # broadcast subs (Pool tensor_scalar/TensorTensor-max are slow
            # or rejected), then one DVE max each
            d_t = rtile("d_t", TS)
            nc.gpsimd.tensor_sub(d_t[:], ot4.unsqueeze(3).broadcast_to(TS),
                                 tt4.unsqueeze(2).broadcast_to(TS))
            d_tn = rtile("d_tn", TS)
            nc.gpsimd.tensor_sub(d_tn[:], tt4.unsqueeze(2).broadcast_to(TS),
                                 ot4.unsqueeze(3).broadcast_to(TS))
            d_a = rtile("d_a", TS)
            nc.gpsimd.tensor_sub(d_a[:], oa4.unsqueeze(3).broadcast_to(TS),
                                 at4.unsqueeze(2).broadcast_to(TS))
            d_an = rtile("d_an", TS)
            nc.gpsimd.tensor_sub(d_an[:], at4.unsqueeze(2).broadcast_to(TS),
                                 oa4.unsqueeze(3).broadcast_to(TS))
            nc.vector.tensor_max(d_t[:], d_t[:], d_tn[:])
            nc.vector.tensor_max(d_a[:], d_a[:], d_an[:])
            base = rtile("base", TS)
            nc.gpsimd.tensor_add(base[:], d_t[:], d_a[:])
            nc.gpsimd.tensor_sub(base[:], base[:], olt)

            # pair sums A[t0,t1] = base[k0,t0]+base[k1,t1] (B for k2,k3)
            A = rtile("A", TS)
            nc.gpsimd.tensor_add(A[:], base[:, :, 0, :].unsqueeze(3).broadcast_to(TS),
                                 base[:, :, 1, :].unsqueeze(2).broadcast_to(TS))
            Bp = rtile("Bp", TS)
            nc.gpsimd.tensor_add(Bp[:], base[:, :, 2, :].unsqueeze(3).broadcast_to(TS),
                                 base[:, :, 3, :].unsqueeze(2).broadcast_to(TS))
            pmin = rtile("pmin", (P, NT), f32)

            def emit_perm_min():
                # emitted mid-chunk-loop: DVE executes its queue in order, so
                # this lands when A/Bp (GpSimd) are long done, without
                # blocking the reduction stream behind a sem wait
                mA = rtile("mA", TS)
                nc.vector.tensor_tensor(out=mA[:], in0=A[:],
                                        in1=A[:].transpose([0, 1, 3, 2]),
                                        op=ALU.min)
                mB = rtile("mB", TS)
                nc.vector.tensor_tensor(out=mB[:], in0=Bp[:],
                                        in1=Bp[:].transpose([0, 1, 3, 2]),
                                        op=ALU.min)
                V6 = rtile("V6", (P, NT, 6))
                for q, (ja, jb) in enumerate(SPLITS):
                    a0, a1 = divmod(ja, 4)
                    b0, b1 = divmod(jb, 4)
                    nc.vector.tensor_add(V6[:, :, q], mA[:, :, a0, a1],
                                         mB[:, :, b0, b1])
                nc.vector.tensor_reduce(out=pmin[:], in_=V6[:], axis=AX,
                                        op=ALU.min)

            # leftover pieces (tiny): e4 = exp(ps)+1, qe = prod_k e4,
            # pss = sum_k ps
            e4 = rtile("e4", (P, NT, K))
            nc.scalar.activation(out=e4[:], in_=ps4, func=AF.Exp)
            nc.vector.tensor_add(e4[:], e4[:],
                                 ones[:].unsqueeze(2).broadcast_to((P, NT, K)))
            q1 = rtile("q1", (P, NT, 2))
            nc.vector.tensor_mul(q1[:], e4[:, :, 0:2], e4[:, :, 2:4])
            qe = rtile("qe", (P, NT), f32)
            nc.vector.tensor_mul(qe[:], q1[:, :, 0], q1[:, :, 1])
            pss = rtile("pss", (P, NT), f32)
            nc.vector.tensor_reduce(out=pss[:], in_=ps4, axis=AX, op=ALU.add)

            # --- sum(exp) over C per (window, k): fp16 halving-tree adds
            # (packed tensor_tensor = 2x on DVE) + short reduce; graduated
            # chunks; per-chunk product over k folded into the stream ---
            se_all = rtile("se_all", (P, NT, K))
            qs = rtile("qs", (P, NT), f32)
            off = 0
            for ci, t in enumerate(CHUNKS):
                cw = t * KC
                nb = sum(1 for x in CHUNKS if x == t)
                lg = big.tile([P, cw], f16, tag=f"lg{t}", name=f"lg{ci}", bufs=nb)
                nc.sync.dma_start(out=lg[:], in_=lg_d.ap()[:, off * KC:off * KC + cw])
                nc.scalar.activation(out=lg[:], in_=lg[:], func=AF.Exp)
                g = t * K
                v = lg[:].rearrange("p (g c) -> p g c", c=C)
                h1 = big.tile([P, g, 64], f16, tag=f"h1{t}", name=f"h1_{ci}", bufs=nb)
                nc.vector.tensor_add(h1[:], v[:, :, 0:64], v[:, :, 64:128])
                h2 = big.tile([P, g, 32], f16, tag=f"h2{t}", name=f"h2_{ci}", bufs=nb)
                nc.vector.tensor_add(h2[:], h1[:, :, 0:32], h1[:, :, 32:64])
                h3 = big.tile([P, g, 16], f16, tag=f"h3{t}", name=f"h3_{ci}", bufs=nb)
                nc.vector.tensor_add(h3[:], h2[:, :, 0:16], h2[:, :, 16:32])
                with nc.allow_low_precision(reason="sumexp fits fp16"):
                    nc.vector.tensor_reduce(
                        out=se_all[:, off:off + t, :], in_=h3[:],
                        axis=AX, op=ALU.add)
                # qs slice: prod_k sumexp for this chunk's windows
                nc.vector.tensor_reduce(
                    out=qs[:, off:off + t],
                    in_=se_all[:, off:off + t, :], axis=AX, op=ALU.mult)
                off += t
                if ci == 4:
                    emit_perm_min()

            # ln(prod_k se_k * prod_k (1+e^ps_k)) = sum_k lse_k + softplus sum
            qq = rtile("qq", (P, NT), f32)
            nc.vector.tensor_mul(qq[:], qs[:], qe[:])
            lnq = rtile("lnq", (P, NT), f32)
            nc.scalar.activation(out=lnq[:], in_=qq[:], func=AF.Ln)

            # --- total = (pmin + lnq - pss) * m; partial sums per partition ---
            tot = rtile("tot", (P, NT), f32)
            nc.vector.tensor_add(tot[:], pmin[:], lnq[:])
            nc.vector.tensor_sub(tot[:], tot[:], pss[:])
            totm = rtile("totm", (P, NT), f32)
            nc.vector.tensor_mul(totm[:], tot[:], m1)
            rowsum = rtile("rowsum", (P, 1), f32)
            nc.vector.tensor_reduce(out=rowsum[:], in_=totm[:], axis=AX, op=ALU.add)
            nc.sync.dma_start(out=out_d.ap(), in_=rowsum[:])

    nc.compile()
    return nc


def _get_program():
    global _PROGRAM
    if _PROGRAM is None:
        _PROGRAM = _build_program()
    return _PROGRAM


def kernel(**inputs):
    g = _prep(**inputs)
    in_maps = [_pack_core(g, d) for d in range(NCORES)]
    nc = _get_program()
    from concourse.bass_utils import run_bass_kernel_spmd
    res = run_bass_kernel_spmd(nc, in_maps, list(range(NCORES)))
    total = sum(r["partial"].sum(dtype=np.float64) for r in res.results)
    V = g["m"].sum(dtype=np.float64)
    return np.asarray(np.float32(total) / np.float32(V))
